# revision 24
# baseline (speedup 1.0000x reference)
"""Trainium2 Bass kernel for nn_ConvHDC (binary HDC conv encoder + classifier).

v4 — field-layout conv1 + fp8 DoubleRow conv2 + host-side weight prep
(baseline 117.0us -> 88us):

Sharding: D=10000 padded to 10240 -> 1280 channels/core across 8 cores
(depthwise after conv1 => fully local); per-core partial [16,10] sims are
summed on the host (no device collective => no cross-core barrier).

Key points per core:
  layout : conv1 output columns are stored in "field" order
           (h, w-parity, w//2, batch) with batch innermost, 2912 cols
           incl. 208 never-read pads.  This makes every conv2 tap window
           a 3-free-dim AP [pair][oh][ow*b] so tap-paired DoubleRow
           matmuls are legal (walrus limit: 3 free dims).
  conv1  : im2col patches split hi/lo into two bf16 streams STACKED on
           the contraction dim (K=50) so each 512-col chunk is ONE
           plain bf16 matmul at 1 cy/col — exact to 2^-16 and immune to
           the reduced-precision DoubleRow accumulation tree (measured
           2e-3 abs err, which flips h2 lattice channels).
  BN1    : mean and E[x^2] from the 25x25 patch covariance via one
           K-stacked matmul per tile (3 bf16 parts stacked to K=75),
           then batched DVE algebra.
  conv2  : depthwise 3x3/s2 as 5 DoubleRow fp8 matmuls per group — tap
           pairs with diagonalized per-channel weights built on the
           HOST and DMA'd per tile (no gpsimd affine_select).  DR is
           exact here: products are ±1 with <=2 nonzeros per column.
  conv3  : depthwise 6x6 -> 1 via gpsimd multiply + DVE reduce.
  signs of W1/W2/Wc are taken on the host; conv biases b1/b2/b3 are
  dropped (training-mode BN is invariant to per-channel pre-bias).
  conv1 psum is split into three ping-ponged 2-bank thirds (pc1 bufs=2,
  4 banks) which frees 4 banks for a double-buffered conv2 psum pool —
  conv2(t+1) then never waits on tile t's stats/is_gt chain, keeping the
  PE queue busy; input DMAs are spread over the sync and gpsimd queues.
"""

import sys

if "/opt/trn_rl_repo" not in sys.path:
    sys.path.insert(0, "/opt/trn_rl_repo")

import numpy as np
from numpy.lib.stride_tricks import sliding_window_view

from concourse import bacc, bass, tile, mybir

F32 = mybir.dt.float32
BF16 = mybir.dt.bfloat16
FP8 = mybir.dt.float8e4
FP8W = mybir.dt.float8e5
ALU = mybir.AluOpType
ACTF = mybir.ActivationFunctionType
AX = mybir.AxisListType
DR = mybir.MatmulPerfMode.DoubleRow

NCORES = 8
D = 10000
DPAD = 10240
DP = DPAD // NCORES          # 1280 channels per core
DT = DP // 128               # 10 tiles of 128 channels
B = 16
EPS = 1e-5

H1 = 13
N1 = B * H1 * H1             # 2704 real conv1 outputs
N1F = B * H1 * 14            # 2912 field-layout cols (with pads)
# conv1 psum thirds: 3 ping-ponged 2-bank tiles (pc1 bufs=2 -> 4 banks),
# leaving 4 banks for double-buffered pc2
CH3 = [[512, 512], [512, 512], [512, 352]]
G2 = 288                     # conv2 psum group = 3 oh-rows x 6 ow x 16 b

# conv2 tap pairs in field layout: (tap_a, tap_b, base offset, delta)
# field strides per channel: h:224, parity:112, wp:16, b:1
PAIRS = [(0, 2, 0, 16), (1, 3, 112, 112), (5, 4, 240, 96),
         (6, 8, 448, 16), (7, None, 560, 0)]

_CACHE = {}


def _pair_ap(h1b, gg, base, delta):
    """rhs AP [128, 2(pair), 3(oh), 96(ow*b)] into h1b [128, N1F]."""
    v = h1b[:]
    ap = [list(v.ap[0]), [delta, 2], [448, 3], [1, 96]]
    return bass.AP(v.tensor, v.offset + gg * 1344 + base, ap)


def _build_bass():
    nc = bacc.Bacc("TRN2", target_bir_lowering=False, debug=False,
                   num_devices=NCORES)

    pext_d = nc.dram_tensor("pext", [50, N1F], BF16, kind="ExternalInput").ap()
    w1pr_d = nc.dram_tensor("w1pr", [50, DP], BF16, kind="ExternalInput").ap()
    cub_d = nc.dram_tensor("cub", [75, 32], BF16, kind="ExternalInput").ap()
    w1tr_d = nc.dram_tensor("w1tr", [75, DP], FP8, kind="ExternalInput").ap()
    w1p_d = nc.dram_tensor("w1p", [128, DT * 32], BF16, kind="ExternalInput").ap()
    diag_d = nc.dram_tensor("diag", [128, DT * 5 * 2 * 128], FP8,
                            kind="ExternalInput").ap()
    w3_d = nc.dram_tensor("w3", [128, DT * 36], F32, kind="ExternalInput").ap()
    bn_d = nc.dram_tensor("bn", [128, 60], F32, kind="ExternalInput").ap()
    wct_d = nc.dram_tensor("wct", [128, DT * 10], BF16, kind="ExternalInput").ap()
    out_d = nc.dram_tensor("sims", [B, 10], F32, kind="ExternalOutput").ap()

    with tile.TileContext(nc) as tc:
        with (
            tc.tile_pool(name="const", bufs=1) as const,
            tc.tile_pool(name="work", bufs=3) as work,
            tc.tile_pool(name="stat", bufs=2) as stat,
            tc.tile_pool(name="pc1", bufs=2, space="PSUM") as pc1,
            tc.tile_pool(name="pc2", bufs=2, space="PSUM") as pc2,
        ):
            # ---------------- DMA in (spread across engine queues so
            # the rings + transfers overlap; every engine is idle at t=0).
            # scalar queue: the small tensors gating the V phase + bias1;
            # sync/gpsimd: the two pext halves + w1pr + diag.
            cub = const.tile([75, 32], BF16)
            nc.sync.dma_start(out=cub[:], in_=cub_d[:])
            w1tr = const.tile([75, DP], FP8)
            nc.sync.dma_start(out=w1tr[:], in_=w1tr_d[:])
            w1pr = const.tile([50, DP], BF16)
            nc.sync.dma_start(out=w1pr[:], in_=w1pr_d[:])
            pext = const.tile([50, N1F], BF16)
            nc.sync.dma_start(out=pext[:, 0:1536], in_=pext_d[:, 0:1536])
            nc.gpsimd.dma_start(out=pext[:, 1536:N1F],
                                in_=pext_d[:, 1536:N1F])
            bnt = const.tile([128, 6, DT], F32)
            nc.scalar.dma_start(
                out=bnt[:].rearrange("p l t -> p (l t)"), in_=bn_d[:])
            w1p = const.tile([128, DT, 32], BF16)
            nc.scalar.dma_start(
                out=w1p[:].rearrange("p t k -> p (t k)"), in_=w1p_d[:])
            diag = const.tile([128, DT, 5, 2, 128], FP8)
            DTL = 5 * 2 * 128
            for t in range(DT):
                nc.gpsimd.dma_start(
                    out=diag[:, t].rearrange("p k i c -> p (k i c)"),
                    in_=diag_d[:, t * DTL:(t + 1) * DTL])
            w3t = const.tile([128, DT, 36], F32)
            nc.gpsimd.dma_start(
                out=w3t[:].rearrange("p t s -> p (t s)"), in_=w3_d[:])
            wcs = const.tile([128, DT, 10], BF16)
            nc.gpsimd.dma_start(
                out=wcs[:].rearrange("p t c -> p (t c)"), in_=wct_d[:])

            epsc = const.tile([128, 1], F32)
            nc.vector.memset(epsc[:], EPS)

            # beMrgT[:, L, t] = (beta_L - 0.5) / gamma_L   (gamma > 0)
            rg = const.tile([128, 3, DT], F32)
            nc.vector.reciprocal(rg[:], bnt[:, 0::2, :])
            beMrgT = const.tile([128, 3, DT], F32)
            nc.vector.tensor_scalar(beMrgT[:], bnt[:, 1::2, :], -0.5, None,
                                    ALU.add)
            nc.vector.tensor_tensor(beMrgT[:], beMrgT[:], rg[:], ALU.mult)
            negBeMrgT = const.tile([128, 3, DT], F32)
            nc.vector.tensor_scalar(negBeMrgT[:], beMrgT[:], -1.0, None,
                                    ALU.mult)

            h3b_all = const.tile([128, DT, B], BF16)
            bias1 = const.tile([128, DT], F32)

            # ---------------- pipelined tile loop (defs) ----------------
            h1bs = {}

            def conv1_mm_thunks(t):
                lhs = w1pr[:, t * 128:(t + 1) * 128]
                thirds = []
                mms = []
                o = 0
                for chunks in CH3:
                    p1 = pc1.tile([128, 2, 512], F32, tag="c1")
                    for ci, cs in enumerate(chunks):
                        def mk(p1=p1, ci=ci, cs=cs, o=o):
                            nc.tensor.matmul(p1[:, ci, 0:cs], lhsT=lhs,
                                             rhs=pext[:, o:o + cs],
                                             start=True, stop=True)
                        mms.append(mk)
                        o += cs
                    thirds.append(p1)
                return thirds, mms

            def conv1_mm(t):
                thirds, mms = conv1_mm_thunks(t)
                for fn in mms:
                    fn()
                return thirds

            def conv1_bin(t, thirds):
                # h1b padded to 3072: each psum third binarizes in ONE op
                # (cols beyond 2912 are junk, never read by conv2; the last
                # third reads 160 unwritten psum cols whose junk lands in
                # the junk zone).  Odd tiles binarize on the DVE as {0,1}
                # (is_gt) — training-mode BN2 absorbs the per-channel
                # affine difference — halving the ACT queue latency that
                # gates conv1(t+1) and conv2(t).
                h1b = work.tile([128, 3072], FP8, tag="h1b")
                for k, p1 in enumerate(thirds):
                    dst = h1b[:, k * 1024:(k + 1) * 1024].rearrange(
                        "p (a b) -> p a b", a=2, b=512)
                    if t % 2 == 0:
                        nc.scalar.activation(dst, p1[:], ACTF.Sign,
                                             bias=bias1[:, t:t + 1], scale=1.0)
                    else:
                        nc.vector.tensor_scalar(dst, p1[:],
                                                negb1[:, t:t + 1], None,
                                                ALU.is_gt)
                h1bs[t] = h1b

            # ---------------- phase V: BN1 stats matmuls ----------------
            # V[c, t*32+k] = w1s_t^T @ [C|u] with the 3 bf16 parts stacked
            # on K (75 rows); one 32-col matmul per tile into bank 0.
            pv = pc2.tile([128, 2, 512], F32, tag="c2")
            pvv = pv[:, 0, 0:DT * 32].rearrange("p (t k) -> p t k", t=DT, k=32)
            for t in range(DT):
                nc.tensor.matmul(pvv[:, t, :],
                                 lhsT=w1tr[:, t * 128:(t + 1) * 128],
                                 rhs=cub[:], start=True, stop=True)

            # conv1(0) matmuls after the V phase on the PE queue: V's
            # inputs (cub/w1tr, scalar queue) land before pext, so the PE
            # starts on V while pext streams in.
            thirds0 = conv1_mm(0)

            # ---------------- batched BN1 algebra (DVE) ----------------
            H = const.tile([128, DT, 32], F32)
            nc.vector.tensor_tensor(H[:], pvv, w1p[:], ALU.mult)
            m2s = const.tile([128, DT], F32)
            nc.vector.tensor_reduce(m2s[:], H[:], AX.X, ALU.add)
            mean1 = const.tile([128, DT], F32)
            nc.vector.tensor_scalar(mean1[:], pvv[:, :, 25], 1.0 / N1,
                                    None, ALU.mult)
            mm1 = const.tile([128, DT], F32)
            nc.vector.tensor_tensor(mm1[:], mean1[:], mean1[:], ALU.mult)
            var1 = const.tile([128, DT], F32)
            nc.vector.scalar_tensor_tensor(var1[:], m2s[:], 1.0 / N1, mm1[:],
                                           ALU.mult, ALU.subtract)
            sd1 = const.tile([128, DT], F32)
            nc.scalar.activation(sd1[:], var1[:], ACTF.Sqrt, bias=epsc[:],
                                 scale=1.0)
            nc.vector.tensor_tensor(bias1[:], sd1[:], beMrgT[:, 0, :],
                                    ALU.mult)
            nc.vector.tensor_tensor(bias1[:], bias1[:], mean1[:], ALU.subtract)
            negb1 = const.tile([128, DT], F32)
            nc.vector.tensor_scalar(negb1[:], bias1[:], -1.0, None, ALU.mult)

            conv1_bin(0, thirds0)

            # ---------------- pipelined tile loop ----------------
            def conv2_front(t, c1_mms=None):
                """Gating path: conv2 matmuls -> stats -> thr2 -> is_gt.

                conv2 is LDWEIGHTS-bound (229ns load vs 120ns exec); conv1
                is exec-bound (427ns).  Interleaving conv1(t+1) chunks
                between conv2(t) pairs hides every conv2 weight load
                behind a conv1 exec on the in-order PE queue."""
                h1b = h1bs.pop(t)
                c1_mms = list(c1_mms or [])
                p2 = pc2.tile([128, 2, 512], F32, tag="c2")
                for gg in range(2):
                    for pk, (ka, kb, base, delta) in enumerate(PAIRS):
                        if c1_mms:
                            c1_mms.pop(0)()
                        nc.tensor.matmul(p2[:, gg, 0:G2],
                                         lhsT=diag[:, t, pk],
                                         rhs=_pair_ap(h1b, gg, base, delta),
                                         start=(pk == 0), stop=(pk == 4),
                                         perf_mode=DR)
                for fn in c1_mms:
                    fn()
                # the whole stats->threshold->is_gt chain gates conv2(t+1)
                # via the p2 buffer; raise its scheduler priority so it
                # sorts ahead of the previous tile's conv3 tail ops.
                with tc.high_priority(offset=25):
                    st2 = stat.tile([128, 2, 6], F32, tag="st2")
                    for gg in range(2):
                        nc.vector.bn_stats(st2[:, gg, :], p2[:, gg, 0:G2])
                    mv2 = stat.tile([128, 2], F32, tag="mv2")
                    nc.vector.bn_aggr(mv2[:], st2[:])
                    sq2 = stat.tile([128, 1], F32, tag="sq2")
                    nc.scalar.activation(sq2[:], mv2[:, 1:2], ACTF.Sqrt,
                                         bias=epsc[:], scale=1.0)
                    # threshold: h2 = (y > thr2) in {0,1}; BN3 absorbs the
                    # per-channel affine change of conv3 preacts.
                    thr2 = stat.tile([128, 1], F32, tag="thr2")
                    nc.vector.scalar_tensor_tensor(
                        thr2[:], sq2[:], negBeMrgT[:, 1, t:t + 1],
                        mv2[:, 0:1], ALU.mult, ALU.add)
                    h2b = work.tile([128, 2, G2], BF16, tag="h2b")
                    nc.vector.tensor_scalar(h2b[:], p2[:, :, 0:G2], thr2[:],
                                            None, ALU.is_gt)
                return h2b

            def conv3_tail(t, h2b):
                """Lagging path: conv3 + BN3 + binarize3 (not tile-gating)."""
                h2sb = h2b[:].rearrange(
                    "p g (x b) -> p g x b", b=B).transpose([0, 3, 1, 2])
                tmp3 = work.tile([128, B, 36], F32, tag="tmp3")
                mul_eng = nc.vector if t == DT - 1 else nc.gpsimd
                mul_eng.tensor_tensor(
                    tmp3[:].rearrange("p b (g x) -> p b g x", g=2, x=18), h2sb,
                    w3t[:, t, :].unsqueeze(1).broadcast_to(
                        [128, B, 36]).rearrange("p b (g x) -> p b g x",
                                                g=2, x=18),
                    ALU.mult)
                h3pre = stat.tile([128, B], F32, tag="h3pre")
                nc.vector.tensor_reduce(h3pre[:], tmp3[:], AX.X, ALU.add)
                st3 = stat.tile([128, 6], F32, tag="st3")
                nc.vector.bn_stats(st3[:], h3pre[:])
                mv3 = stat.tile([128, 2], F32, tag="mv3")
                nc.vector.bn_aggr(mv3[:], st3[:])
                sq3 = stat.tile([128, 1], F32, tag="sq3")
                nc.scalar.activation(sq3[:], mv3[:, 1:2], ACTF.Sqrt,
                                     bias=epsc[:], scale=1.0)
                bias3 = stat.tile([128, 1], F32, tag="bias3")
                nc.vector.scalar_tensor_tensor(
                    bias3[:], sq3[:], beMrgT[:, 2, t:t + 1], mv3[:, 0:1],
                    ALU.mult, ALU.subtract)
                nc.scalar.activation(h3b_all[:, t, :], h3pre[:], ACTF.Sign,
                                     bias=bias3[:], scale=1.0)

            # Emission order per tile:
            #   c1mm(t+1)  [PE ahead of c2mm(t)]
            #   c2front(t) [gating: mm, stats, sq2, thr2, is_gt]
            #   c1bin(t+1) [ACT right after sq2(t), before sq3(t)/h3(t)]
            #   c3tail(t)  [lagging chain, never gates tile t+1]
            for it in range(DT):
                nxt, mms = (None, None)
                if it + 1 < DT:
                    nxt, mms = conv1_mm_thunks(it + 1)
                h2b = conv2_front(it, mms)
                if nxt is not None:
                    conv1_bin(it + 1, nxt)
                conv3_tail(it, h2b)

            # ---------------- classifier (partial sims per core) ----------
            pcls = pc2.tile([128, 2, 512], F32, tag="c2")
            for t in range(DT):
                nc.tensor.matmul(pcls[0:B, 0, 0:10], lhsT=h3b_all[:, t, :],
                                 rhs=wcs[:, t, :],
                                 start=(t == 0), stop=(t == DT - 1))
            sims_sb = stat.tile([B, 10], F32, tag="sims_sb")
            nc.scalar.mul(sims_sb[:], pcls[0:B, 0, 0:10],
                          1.0 / np.sqrt(np.float32(D)))
            nc.sync.dma_start(out=out_d[:], in_=sims_sb[:])

    nc.compile()
    return nc


def get_nc():
    if "nc" not in _CACHE:
        _CACHE["nc"] = _build_bass()
    return _CACHE["nc"]


def prep_inputs(x, W1, b1, g1, be1, W2, b2, g2, be2, W3, b3, g3, be3, Wc):
    """Host-side layout/sharding prep.

    Conv biases b1/b2/b3 are dropped: training-mode BN is invariant to a
    per-channel additive constant before normalization.
    """
    import ml_dtypes
    f = np.float32
    bf = ml_dtypes.bfloat16
    f8 = ml_dtypes.float8_e4m3
    f8w = ml_dtypes.float8_e5m2

    xp = np.zeros((B, 30, 30), f)
    xp[:, 1:29, 1:29] = np.asarray(x, f)[:, 0]
    win = sliding_window_view(xp, (5, 5), axis=(1, 2))[:, ::2, ::2]
    # field-order columns [h][w-parity][w//2][b] (batch innermost, strides
    # h:224 par:112 wp:16 b:1) with 1 pad col-group per h row
    Pf = np.zeros((25, H1, 2, 7, B), f)
    wv = win.transpose(3, 4, 0, 1, 2)          # [5,5,b,h,w] -> flat 25
    wv = wv.reshape(25, B, H1, H1)
    Pf[:, :, 0, :, :] = wv[:, :, :, 0::2].transpose(0, 2, 3, 1)  # w even (7)
    Pf[:, :, 1, :6, :] = wv[:, :, :, 1::2].transpose(0, 2, 3, 1)  # w odd (6)
    P = Pf.reshape(25, N1F)

    # hi/lo bf16 streams stacked on K (exact to 2^-16)
    phi = P.astype(bf)
    plo = (P - phi.astype(f)).astype(bf)
    pext = np.zeros((50, N1F), bf)
    pext[0:25] = phi
    pext[25:50] = plo

    # covariance+colsum in 3 bf16 parts stacked on K
    P64 = P.astype(np.float64)
    cu32 = np.concatenate(
        [(P64 @ P64.T).astype(f), P64.sum(1).astype(f)[:, None]], 1)  # [25,26]
    cub = np.zeros((75, 32), bf)
    rem2 = cu32.astype(np.float64)
    for part in range(3):
        p_ = rem2.astype(f).astype(bf)
        cub[part * 25:(part + 1) * 25, 0:26] = p_
        rem2 = rem2 - p_.astype(np.float64)

    def padrows(a, width, fill=0.0):
        out = np.full((DPAD, width), fill, f)
        out[:D] = np.asarray(a, f).reshape(D, width)
        return out

    w1sp = np.sign(padrows(W1, 25))          # [DPAD, 25] in {-1,0,1}
    w2sp = np.sign(padrows(W2, 9))
    w3p = padrows(W3, 36)
    wcsp = np.sign(padrows(np.asarray(Wc, f).T, 10))
    bn = np.zeros((DPAD, 6), f)
    bn[:, 0::2] = 1.0
    bn[:D, 0] = np.asarray(g1, f)
    bn[:D, 1] = np.asarray(be1, f)
    bn[:D, 2] = np.asarray(g2, f)
    bn[:D, 3] = np.asarray(be2, f)
    bn[:D, 4] = np.asarray(g3, f)
    bn[:D, 5] = np.asarray(be3, f)

    def tmajor(a, width):
        # [1280, width] -> [128, DT, width]
        return np.ascontiguousarray(
            a.reshape(DT, 128, width).transpose(1, 0, 2))

    in_maps = []
    for c in range(NCORES):
        sl = slice(c * DP, (c + 1) * DP)
        w1sl = w1sp[sl]                       # [1280, 25]
        w1t = w1sl.T                          # [25, 1280]
        # bf16 K-stacked (2x) conv1 weights
        w1pr = np.zeros((50, DP), bf)
        w1pr[0:25] = w1t.astype(bf)
        w1pr[25:50] = w1t.astype(bf)
        # K-stacked (3x) fp8 weights for the BN1 stats matmuls
        w1tr = np.zeros((75, DP), f8)
        for j in range(3):
            w1tr[j * 25:(j + 1) * 25] = w1t.astype(f8)
        # w1s t-major padded to 32 for the H product
        w1pt = np.zeros((128, DT, 32), bf)
        w1pt[:, :, 0:25] = tmajor(w1sl, 25)
        # conv2 diagonal pair weights
        w2tm = tmajor(w2sp[sl], 9)            # [128, DT, 9]
        diagp = np.zeros((128, DT, 5, 2, 128), f8)
        idx = np.arange(128)
        for pk, (ka, kb, base, delta) in enumerate(PAIRS):
            diagp[idx, :, pk, 0, idx] = w2tm[:, :, ka].astype(f8)
            if kb is not None:
                diagp[idx, :, pk, 1, idx] = w2tm[:, :, kb].astype(f8)
        in_maps.append({
            "pext": pext,
            "w1pr": w1pr,
            "cub": cub,
            "w1tr": w1tr,
            "w1p": w1pt.reshape(128, DT * 32),
            "diag": diagp.reshape(128, DT * 5 * 2 * 128),
            "w3": tmajor(w3p[sl], 36).reshape(128, DT * 36),
            # bn: [128, 6, DT] L-major, t-minor
            "bn": np.ascontiguousarray(
                tmajor(bn[sl], 6).transpose(0, 2, 1)).reshape(128, 60),
            "wct": tmajor(wcsp[sl], 10).astype(bf).reshape(128, DT * 10),
        })
    return in_maps


def kernel(**inputs) -> np.ndarray:
    from concourse.bass_utils import run_bass_kernel_spmd
    nc = get_nc()
    in_maps = prep_inputs(**inputs)
    res = run_bass_kernel_spmd(nc, in_maps, list(range(NCORES)))
    acc = np.zeros((B, 10), np.float64)
    for r in res.results:
        acc += np.asarray(r["sims"], np.float64)
    return acc.astype(np.float32)


# revision 26
# speedup vs baseline: 1.0198x; 1.0198x over previous
"""Trainium2 Bass kernel for nn_ConvHDC (binary HDC conv encoder + classifier).

v4 — field-layout conv1 + fp8 DoubleRow conv2 + host-side weight prep
(baseline 117.0us -> 88us):

Sharding: D=10000 padded to 10240 -> 1280 channels/core across 8 cores
(depthwise after conv1 => fully local); per-core partial [16,10] sims are
summed on the host (no device collective => no cross-core barrier).

Key points per core:
  layout : conv1 output columns are stored in "field" order
           (h, w-parity, w//2, batch) with batch innermost, 2912 cols
           incl. 208 never-read pads.  This makes every conv2 tap window
           a 3-free-dim AP [pair][oh][ow*b] so tap-paired DoubleRow
           matmuls are legal (walrus limit: 3 free dims).
  conv1  : im2col patches split hi/lo into two bf16 streams STACKED on
           the contraction dim (K=50) so each 512-col chunk is ONE
           plain bf16 matmul at 1 cy/col — exact to 2^-16 and immune to
           the reduced-precision DoubleRow accumulation tree (measured
           2e-3 abs err, which flips h2 lattice channels).
  BN1    : mean and E[x^2] from the 25x25 patch covariance via one
           K-stacked matmul per tile (3 bf16 parts stacked to K=75),
           then batched DVE algebra.
  conv2  : depthwise 3x3/s2 as 5 DoubleRow fp8 matmuls per group — tap
           pairs with diagonalized per-channel weights built on the
           HOST and DMA'd per tile (no gpsimd affine_select).  DR is
           exact here: products are ±1 with <=2 nonzeros per column.
  conv3  : depthwise 6x6 -> 1 via gpsimd multiply + DVE reduce.
  signs of W1/W2/Wc are taken on the host; conv biases b1/b2/b3 are
  dropped (training-mode BN is invariant to per-channel pre-bias).
  conv1 psum is split into three ping-ponged 2-bank thirds (pc1 bufs=2,
  4 banks) which frees 4 banks for a double-buffered conv2 psum pool —
  conv2(t+1) then never waits on tile t's stats/is_gt chain, keeping the
  PE queue busy; input DMAs are spread over the sync and gpsimd queues.
"""

import sys

if "/opt/trn_rl_repo" not in sys.path:
    sys.path.insert(0, "/opt/trn_rl_repo")

import numpy as np
from numpy.lib.stride_tricks import sliding_window_view

from concourse import bacc, bass, tile, mybir

F32 = mybir.dt.float32
BF16 = mybir.dt.bfloat16
FP8 = mybir.dt.float8e4
FP8W = mybir.dt.float8e5
ALU = mybir.AluOpType
ACTF = mybir.ActivationFunctionType
AX = mybir.AxisListType
DR = mybir.MatmulPerfMode.DoubleRow

NCORES = 8
D = 10000
DPAD = 10240
DP = DPAD // NCORES          # 1280 channels per core
DT = DP // 128               # 10 tiles of 128 channels
B = 16
EPS = 1e-5

H1 = 13
N1 = B * H1 * H1             # 2704 real conv1 outputs
N1F = B * H1 * 14            # 2912 field-layout cols (with pads)
# conv1 psum: six single-bank chunks in a 4-deep ring (pc1 bufs=4 ->
# 4 banks), leaving 4 banks for double-buffered pc2
CH6 = [512, 512, 512, 512, 512, 352]
G2 = 288                     # conv2 psum group = 3 oh-rows x 6 ow x 16 b

# conv2 tap pairs in field layout: (tap_a, tap_b, base offset, delta)
# field strides per channel: h:224, parity:112, wp:16, b:1
PAIRS = [(0, 2, 0, 16), (1, 3, 112, 112), (5, 4, 240, 96),
         (6, 8, 448, 16), (7, None, 560, 0)]

_CACHE = {}


def _pair_ap(h1b, gg, base, delta):
    """rhs AP [128, 2(pair), 3(oh), 96(ow*b)] into h1b [128, N1F]."""
    v = h1b[:]
    ap = [list(v.ap[0]), [delta, 2], [448, 3], [1, 96]]
    return bass.AP(v.tensor, v.offset + gg * 1344 + base, ap)


def _build_bass():
    nc = bacc.Bacc("TRN2", target_bir_lowering=False, debug=False,
                   num_devices=NCORES)

    pext_d = nc.dram_tensor("pext", [50, N1F], BF16, kind="ExternalInput").ap()
    w1pr_d = nc.dram_tensor("w1pr", [50, DP], BF16, kind="ExternalInput").ap()
    cub_d = nc.dram_tensor("cub", [75, 32], BF16, kind="ExternalInput").ap()
    w1tr_d = nc.dram_tensor("w1tr", [75, DP], FP8, kind="ExternalInput").ap()
    w1p_d = nc.dram_tensor("w1p", [128, DT * 32], BF16, kind="ExternalInput").ap()
    diag_d = nc.dram_tensor("diag", [128, DT * 5 * 2 * 128], FP8,
                            kind="ExternalInput").ap()
    w3_d = nc.dram_tensor("w3", [128, DT * 36], F32, kind="ExternalInput").ap()
    bn_d = nc.dram_tensor("bn", [128, 60], F32, kind="ExternalInput").ap()
    wct_d = nc.dram_tensor("wct", [128, DT * 10], BF16, kind="ExternalInput").ap()
    out_d = nc.dram_tensor("sims", [B, 10], F32, kind="ExternalOutput").ap()

    with tile.TileContext(nc) as tc:
        with (
            tc.tile_pool(name="const", bufs=1) as const,
            tc.tile_pool(name="work", bufs=3) as work,
            tc.tile_pool(name="stat", bufs=2) as stat,
            tc.tile_pool(name="pc1", bufs=4, space="PSUM") as pc1,
            tc.tile_pool(name="pc2", bufs=2, space="PSUM") as pc2,
        ):
            # ---------------- DMA in (spread across engine queues so
            # the rings + transfers overlap; every engine is idle at t=0).
            # scalar queue: the small tensors gating the V phase + bias1;
            # sync/gpsimd: the two pext halves + w1pr + diag.
            cub = const.tile([75, 32], BF16)
            nc.sync.dma_start(out=cub[:], in_=cub_d[:])
            w1tr = const.tile([75, DP], FP8)
            nc.sync.dma_start(out=w1tr[:], in_=w1tr_d[:])
            w1pr = const.tile([50, DP], BF16)
            nc.sync.dma_start(out=w1pr[:], in_=w1pr_d[:])
            pext = const.tile([50, N1F], BF16)
            nc.sync.dma_start(out=pext[:, 0:1536], in_=pext_d[:, 0:1536])
            nc.gpsimd.dma_start(out=pext[:, 1536:N1F],
                                in_=pext_d[:, 1536:N1F])
            bnt = const.tile([128, 6, DT], F32)
            nc.scalar.dma_start(
                out=bnt[:].rearrange("p l t -> p (l t)"), in_=bn_d[:])
            w1p = const.tile([128, DT, 32], BF16)
            nc.scalar.dma_start(
                out=w1p[:].rearrange("p t k -> p (t k)"), in_=w1p_d[:])
            diag = const.tile([128, DT, 5, 2, 128], FP8)
            DTL = 5 * 2 * 128
            for t in range(DT):
                nc.gpsimd.dma_start(
                    out=diag[:, t].rearrange("p k i c -> p (k i c)"),
                    in_=diag_d[:, t * DTL:(t + 1) * DTL])
            w3t = const.tile([128, DT, 36], F32)
            nc.gpsimd.dma_start(
                out=w3t[:].rearrange("p t s -> p (t s)"), in_=w3_d[:])
            wcs = const.tile([128, DT, 10], BF16)
            nc.gpsimd.dma_start(
                out=wcs[:].rearrange("p t c -> p (t c)"), in_=wct_d[:])

            epsc = const.tile([128, 1], F32)
            nc.vector.memset(epsc[:], EPS)

            # beMrgT[:, L, t] = (beta_L - 0.5) / gamma_L   (gamma > 0)
            rg = const.tile([128, 3, DT], F32)
            nc.vector.reciprocal(rg[:], bnt[:, 0::2, :])
            beMrgT = const.tile([128, 3, DT], F32)
            nc.vector.tensor_scalar(beMrgT[:], bnt[:, 1::2, :], -0.5, None,
                                    ALU.add)
            nc.vector.tensor_tensor(beMrgT[:], beMrgT[:], rg[:], ALU.mult)
            negBeMrgT = const.tile([128, 3, DT], F32)
            nc.vector.tensor_scalar(negBeMrgT[:], beMrgT[:], -1.0, None,
                                    ALU.mult)

            h3b_all = const.tile([128, DT, B], BF16)
            bias1 = const.tile([128, DT], F32)

            # ---------------- pipelined tile loop (defs) ----------------
            h1bs = {}

            def conv1_mm_thunks(t):
                lhs = w1pr[:, t * 128:(t + 1) * 128]
                thirds = []
                mms = []
                o = 0
                for cs in CH6:
                    p1 = pc1.tile([128, 512], F32, tag="c1")
                    def mk(p1=p1, cs=cs, o=o):
                        nc.tensor.matmul(p1[:, 0:cs], lhsT=lhs,
                                         rhs=pext[:, o:o + cs],
                                         start=True, stop=True)
                    mms.append(mk)
                    o += cs
                    thirds.append(p1)
                return thirds, mms

            def conv1_mm(t):
                thirds, mms = conv1_mm_thunks(t)
                for fn in mms:
                    fn()
                return thirds

            def conv1_bin(t, thirds):
                # h1b padded to 3072: each psum third binarizes in ONE ACT
                # (cols beyond 2912 are junk, never read by conv2; the last
                # third reads 160 unwritten psum cols whose sign lands in
                # the junk zone)
                h1b = work.tile([128, 3072], FP8, tag="h1b")
                b1 = bias1[:, t:t + 1]
                for k, p1 in enumerate(thirds):
                    nc.scalar.activation(h1b[:, k * 512:(k + 1) * 512],
                                         p1[:], ACTF.Sign, bias=b1, scale=1.0)
                h1bs[t] = h1b

            # ---------------- phase V: BN1 stats matmuls ----------------
            # V[c, t*32+k] = w1s_t^T @ [C|u] with the 3 bf16 parts stacked
            # on K (75 rows); one 32-col matmul per tile into bank 0.
            pv = pc2.tile([128, 2, 512], F32, tag="c2")
            pvv = pv[:, 0, 0:DT * 32].rearrange("p (t k) -> p t k", t=DT, k=32)
            for t in range(DT):
                nc.tensor.matmul(pvv[:, t, :],
                                 lhsT=w1tr[:, t * 128:(t + 1) * 128],
                                 rhs=cub[:], start=True, stop=True)

            # conv1(0) matmuls after the V phase on the PE queue: V's
            # inputs (cub/w1tr, scalar queue) land before pext, so the PE
            # starts on V while pext streams in.
            thirds0 = conv1_mm(0)

            # ---------------- batched BN1 algebra (DVE) ----------------
            H = const.tile([128, DT, 32], F32)
            nc.vector.tensor_tensor(H[:], pvv, w1p[:], ALU.mult)
            m2s = const.tile([128, DT], F32)
            nc.vector.tensor_reduce(m2s[:], H[:], AX.X, ALU.add)
            mean1 = const.tile([128, DT], F32)
            nc.vector.tensor_scalar(mean1[:], pvv[:, :, 25], 1.0 / N1,
                                    None, ALU.mult)
            mm1 = const.tile([128, DT], F32)
            nc.vector.tensor_tensor(mm1[:], mean1[:], mean1[:], ALU.mult)
            var1 = const.tile([128, DT], F32)
            nc.vector.scalar_tensor_tensor(var1[:], m2s[:], 1.0 / N1, mm1[:],
                                           ALU.mult, ALU.subtract)
            sd1 = const.tile([128, DT], F32)
            nc.scalar.activation(sd1[:], var1[:], ACTF.Sqrt, bias=epsc[:],
                                 scale=1.0)
            nc.vector.tensor_tensor(bias1[:], sd1[:], beMrgT[:, 0, :],
                                    ALU.mult)
            nc.vector.tensor_tensor(bias1[:], bias1[:], mean1[:], ALU.subtract)
            negb1 = const.tile([128, DT], F32)
            nc.vector.tensor_scalar(negb1[:], bias1[:], -1.0, None, ALU.mult)

            conv1_bin(0, thirds0)

            # ---------------- pipelined tile loop ----------------
            def conv2_front(t, c1_mms=None):
                """Gating path: conv2 matmuls -> stats -> thr2 -> is_gt.

                conv2 is LDWEIGHTS-bound (229ns load vs 120ns exec); conv1
                is exec-bound (427ns).  Interleaving conv1(t+1) chunks
                between conv2(t) pairs hides every conv2 weight load
                behind a conv1 exec on the in-order PE queue."""
                h1b = h1bs.pop(t)
                c1_mms = list(c1_mms or [])
                p2 = pc2.tile([128, 2, 512], F32, tag="c2")
                for gg in range(2):
                    for pk, (ka, kb, base, delta) in enumerate(PAIRS):
                        if c1_mms:
                            c1_mms.pop(0)()
                        nc.tensor.matmul(p2[:, gg, 0:G2],
                                         lhsT=diag[:, t, pk],
                                         rhs=_pair_ap(h1b, gg, base, delta),
                                         start=(pk == 0), stop=(pk == 4),
                                         perf_mode=DR)
                for fn in c1_mms:
                    fn()
                # the whole stats->threshold->is_gt chain gates conv2(t+1)
                # via the p2 buffer; raise its scheduler priority so it
                # sorts ahead of the previous tile's conv3 tail ops.
                with tc.high_priority(offset=25):
                    st2 = stat.tile([128, 2, 6], F32, tag="st2")
                    for gg in range(2):
                        nc.vector.bn_stats(st2[:, gg, :], p2[:, gg, 0:G2])
                    mv2 = stat.tile([128, 2], F32, tag="mv2")
                    nc.vector.bn_aggr(mv2[:], st2[:])
                    sq2 = stat.tile([128, 1], F32, tag="sq2")
                    nc.scalar.activation(sq2[:], mv2[:, 1:2], ACTF.Sqrt,
                                         bias=epsc[:], scale=1.0)
                    # threshold: h2 = (y > thr2) in {0,1}; BN3 absorbs the
                    # per-channel affine change of conv3 preacts.
                    thr2 = stat.tile([128, 1], F32, tag="thr2")
                    nc.vector.scalar_tensor_tensor(
                        thr2[:], sq2[:], negBeMrgT[:, 1, t:t + 1],
                        mv2[:, 0:1], ALU.mult, ALU.add)
                    h2b = work.tile([128, 2, G2], BF16, tag="h2b")
                    nc.vector.tensor_scalar(h2b[:], p2[:, :, 0:G2], thr2[:],
                                            None, ALU.is_gt)
                return h2b

            def conv3_tail(t, h2b):
                """Lagging path: conv3 + BN3 + binarize3 (not tile-gating)."""
                h2sb = h2b[:].rearrange(
                    "p g (x b) -> p g x b", b=B).transpose([0, 3, 1, 2])
                tmp3 = work.tile([128, B, 36], F32, tag="tmp3")
                mul_eng = nc.vector if t == DT - 1 else nc.gpsimd
                mul_eng.tensor_tensor(
                    tmp3[:].rearrange("p b (g x) -> p b g x", g=2, x=18), h2sb,
                    w3t[:, t, :].unsqueeze(1).broadcast_to(
                        [128, B, 36]).rearrange("p b (g x) -> p b g x",
                                                g=2, x=18),
                    ALU.mult)
                h3pre = stat.tile([128, B], F32, tag="h3pre")
                nc.vector.tensor_reduce(h3pre[:], tmp3[:], AX.X, ALU.add)
                st3 = stat.tile([128, 6], F32, tag="st3")
                nc.vector.bn_stats(st3[:], h3pre[:])
                mv3 = stat.tile([128, 2], F32, tag="mv3")
                nc.vector.bn_aggr(mv3[:], st3[:])
                sq3 = stat.tile([128, 1], F32, tag="sq3")
                nc.scalar.activation(sq3[:], mv3[:, 1:2], ACTF.Sqrt,
                                     bias=epsc[:], scale=1.0)
                # h3 in {0,1} via DVE is_gt (host corrects the classifier:
                # sims = 2*sims01 - colsum(sign(Wc))/sqrt(D))
                thr3 = stat.tile([128, 1], F32, tag="bias3")
                nc.vector.scalar_tensor_tensor(
                    thr3[:], sq3[:], negBeMrgT[:, 2, t:t + 1], mv3[:, 0:1],
                    ALU.mult, ALU.add)
                nc.vector.tensor_scalar(h3b_all[:, t, :], h3pre[:], thr3[:],
                                        None, ALU.is_gt)

            # Emission order per tile:
            #   c1mm(t+1)  [PE ahead of c2mm(t)]
            #   c2front(t) [gating: mm, stats, sq2, thr2, is_gt]
            #   c1bin(t+1) [ACT right after sq2(t), before sq3(t)/h3(t)]
            #   c3tail(t)  [lagging chain, never gates tile t+1]
            for it in range(DT):
                nxt, mms = (None, None)
                if it + 1 < DT:
                    nxt, mms = conv1_mm_thunks(it + 1)
                h2b = conv2_front(it, mms)
                if nxt is not None:
                    conv1_bin(it + 1, nxt)
                conv3_tail(it, h2b)

            # ---------------- classifier (partial sims per core) ----------
            pcls = pc2.tile([128, 2, 512], F32, tag="c2")
            for t in range(DT):
                nc.tensor.matmul(pcls[0:B, 0, 0:10], lhsT=h3b_all[:, t, :],
                                 rhs=wcs[:, t, :],
                                 start=(t == 0), stop=(t == DT - 1))
            sims_sb = stat.tile([B, 10], F32, tag="sims_sb")
            nc.scalar.mul(sims_sb[:], pcls[0:B, 0, 0:10],
                          1.0 / np.sqrt(np.float32(D)))
            nc.sync.dma_start(out=out_d[:], in_=sims_sb[:])

    nc.compile()
    return nc


def get_nc():
    if "nc" not in _CACHE:
        _CACHE["nc"] = _build_bass()
    return _CACHE["nc"]


def prep_inputs(x, W1, b1, g1, be1, W2, b2, g2, be2, W3, b3, g3, be3, Wc):
    """Host-side layout/sharding prep.

    Conv biases b1/b2/b3 are dropped: training-mode BN is invariant to a
    per-channel additive constant before normalization.
    """
    import ml_dtypes
    f = np.float32
    bf = ml_dtypes.bfloat16
    f8 = ml_dtypes.float8_e4m3
    f8w = ml_dtypes.float8_e5m2

    xp = np.zeros((B, 30, 30), f)
    xp[:, 1:29, 1:29] = np.asarray(x, f)[:, 0]
    win = sliding_window_view(xp, (5, 5), axis=(1, 2))[:, ::2, ::2]
    # field-order columns [h][w-parity][w//2][b] (batch innermost, strides
    # h:224 par:112 wp:16 b:1) with 1 pad col-group per h row
    Pf = np.zeros((25, H1, 2, 7, B), f)
    wv = win.transpose(3, 4, 0, 1, 2)          # [5,5,b,h,w] -> flat 25
    wv = wv.reshape(25, B, H1, H1)
    Pf[:, :, 0, :, :] = wv[:, :, :, 0::2].transpose(0, 2, 3, 1)  # w even (7)
    Pf[:, :, 1, :6, :] = wv[:, :, :, 1::2].transpose(0, 2, 3, 1)  # w odd (6)
    P = Pf.reshape(25, N1F)

    # hi/lo bf16 streams stacked on K (exact to 2^-16)
    phi = P.astype(bf)
    plo = (P - phi.astype(f)).astype(bf)
    pext = np.zeros((50, N1F), bf)
    pext[0:25] = phi
    pext[25:50] = plo

    # covariance+colsum in 3 bf16 parts stacked on K
    P64 = P.astype(np.float64)
    cu32 = np.concatenate(
        [(P64 @ P64.T).astype(f), P64.sum(1).astype(f)[:, None]], 1)  # [25,26]
    cub = np.zeros((75, 32), bf)
    rem2 = cu32.astype(np.float64)
    for part in range(3):
        p_ = rem2.astype(f).astype(bf)
        cub[part * 25:(part + 1) * 25, 0:26] = p_
        rem2 = rem2 - p_.astype(np.float64)

    def padrows(a, width, fill=0.0):
        out = np.full((DPAD, width), fill, f)
        out[:D] = np.asarray(a, f).reshape(D, width)
        return out

    w1sp = np.sign(padrows(W1, 25))          # [DPAD, 25] in {-1,0,1}
    w2sp = np.sign(padrows(W2, 9))
    w3p = padrows(W3, 36)
    wcsp = np.sign(padrows(np.asarray(Wc, f).T, 10))
    bn = np.zeros((DPAD, 6), f)
    bn[:, 0::2] = 1.0
    bn[:D, 0] = np.asarray(g1, f)
    bn[:D, 1] = np.asarray(be1, f)
    bn[:D, 2] = np.asarray(g2, f)
    bn[:D, 3] = np.asarray(be2, f)
    bn[:D, 4] = np.asarray(g3, f)
    bn[:D, 5] = np.asarray(be3, f)

    def tmajor(a, width):
        # [1280, width] -> [128, DT, width]
        return np.ascontiguousarray(
            a.reshape(DT, 128, width).transpose(1, 0, 2))

    in_maps = []
    for c in range(NCORES):
        sl = slice(c * DP, (c + 1) * DP)
        w1sl = w1sp[sl]                       # [1280, 25]
        w1t = w1sl.T                          # [25, 1280]
        # bf16 K-stacked (2x) conv1 weights
        w1pr = np.zeros((50, DP), bf)
        w1pr[0:25] = w1t.astype(bf)
        w1pr[25:50] = w1t.astype(bf)
        # K-stacked (3x) fp8 weights for the BN1 stats matmuls
        w1tr = np.zeros((75, DP), f8)
        for j in range(3):
            w1tr[j * 25:(j + 1) * 25] = w1t.astype(f8)
        # w1s t-major padded to 32 for the H product
        w1pt = np.zeros((128, DT, 32), bf)
        w1pt[:, :, 0:25] = tmajor(w1sl, 25)
        # conv2 diagonal pair weights
        w2tm = tmajor(w2sp[sl], 9)            # [128, DT, 9]
        diagp = np.zeros((128, DT, 5, 2, 128), f8)
        idx = np.arange(128)
        for pk, (ka, kb, base, delta) in enumerate(PAIRS):
            diagp[idx, :, pk, 0, idx] = w2tm[:, :, ka].astype(f8)
            if kb is not None:
                diagp[idx, :, pk, 1, idx] = w2tm[:, :, kb].astype(f8)
        in_maps.append({
            "pext": pext,
            "w1pr": w1pr,
            "cub": cub,
            "w1tr": w1tr,
            "w1p": w1pt.reshape(128, DT * 32),
            "diag": diagp.reshape(128, DT * 5 * 2 * 128),
            "w3": tmajor(w3p[sl], 36).reshape(128, DT * 36),
            # bn: [128, 6, DT] L-major, t-minor
            "bn": np.ascontiguousarray(
                tmajor(bn[sl], 6).transpose(0, 2, 1)).reshape(128, 60),
            "wct": tmajor(wcsp[sl], 10).astype(bf).reshape(128, DT * 10),
        })
    return in_maps


def kernel(**inputs) -> np.ndarray:
    from concourse.bass_utils import run_bass_kernel_spmd
    nc = get_nc()
    in_maps = prep_inputs(**inputs)
    res = run_bass_kernel_spmd(nc, in_maps, list(range(NCORES)))
    acc = np.zeros((B, 10), np.float64)
    for r in res.results:
        acc += 2.0 * np.asarray(r["sims"], np.float64)
    wcs_cols = np.sign(np.asarray(inputs["Wc"], np.float64)).sum(1)
    acc -= wcs_cols[None, :] / np.sqrt(np.float64(D))
    return acc.astype(np.float32)


# revision 27
# speedup vs baseline: 1.1140x; 1.0924x over previous
"""Trainium2 Bass kernel for nn_ConvHDC (binary HDC conv encoder + classifier).

v4 — field-layout conv1 + fp8 DoubleRow conv2 + host-side weight prep
(baseline 117.0us -> 88us):

Sharding: D=10000 padded to 10240 -> 1280 channels/core across 8 cores
(depthwise after conv1 => fully local); per-core partial [16,10] sims are
summed on the host (no device collective => no cross-core barrier).

Key points per core:
  layout : conv1 output columns are stored in "field" order
           (h, w-parity, w//2, batch) with batch innermost, 2912 cols
           incl. 208 never-read pads.  This makes every conv2 tap window
           a 3-free-dim AP [pair][oh][ow*b] so tap-paired DoubleRow
           matmuls are legal (walrus limit: 3 free dims).
  conv1  : im2col patches split hi/lo into two bf16 streams STACKED on
           the contraction dim (K=50) so each 512-col chunk is ONE
           plain bf16 matmul at 1 cy/col — exact to 2^-16 and immune to
           the reduced-precision DoubleRow accumulation tree (measured
           2e-3 abs err, which flips h2 lattice channels).
  BN1    : mean and E[x^2] from the 25x25 patch covariance via one
           K-stacked matmul per tile (3 bf16 parts stacked to K=75),
           then batched DVE algebra.
  conv2  : depthwise 3x3/s2 as 5 DoubleRow fp8 matmuls per group — tap
           pairs with diagonalized per-channel weights built on the
           HOST and DMA'd per tile (no gpsimd affine_select).  DR is
           exact here: products are ±1 with <=2 nonzeros per column.
  conv3  : depthwise 6x6 -> 1 via gpsimd multiply + DVE reduce.
  signs of W1/W2/Wc are taken on the host; conv biases b1/b2/b3 are
  dropped (training-mode BN is invariant to per-channel pre-bias).
  conv1 psum is split into three ping-ponged 2-bank thirds (pc1 bufs=2,
  4 banks) which frees 4 banks for a double-buffered conv2 psum pool —
  conv2(t+1) then never waits on tile t's stats/is_gt chain, keeping the
  PE queue busy; input DMAs are spread over the sync and gpsimd queues.
"""

import sys

if "/opt/trn_rl_repo" not in sys.path:
    sys.path.insert(0, "/opt/trn_rl_repo")

import numpy as np
from numpy.lib.stride_tricks import sliding_window_view

from concourse import bacc, bass, tile, mybir

F32 = mybir.dt.float32
BF16 = mybir.dt.bfloat16
FP8 = mybir.dt.float8e4
FP8W = mybir.dt.float8e5
ALU = mybir.AluOpType
ACTF = mybir.ActivationFunctionType
AX = mybir.AxisListType
DR = mybir.MatmulPerfMode.DoubleRow

NCORES = 8
D = 10000
DPAD = 10240
DP = DPAD // NCORES          # 1280 channels per core
DT = DP // 128               # 10 tiles of 128 channels
B = 16
EPS = 1e-5

H1 = 13
N1 = B * H1 * H1             # 2704 real conv1 outputs
N1F = B * H1 * 14            # 2912 field-layout cols (with pads)
# conv1 psum thirds: 3 ping-ponged 2-bank tiles (pc1 bufs=2 -> 4 banks),
# leaving 4 banks for double-buffered pc2
CH3 = [[512, 512], [512, 512], [512, 352]]
G2 = 288                     # conv2 psum group = 3 oh-rows x 6 ow x 16 b

# conv2 tap pairs in field layout: (tap_a, tap_b, base offset, delta)
# field strides per channel: h:224, parity:112, wp:16, b:1
PAIRS = [(0, 2, 0, 16), (1, 3, 112, 112), (5, 4, 240, 96),
         (6, 8, 448, 16), (7, None, 560, 0)]

_CACHE = {}


def _pair_ap(h1b, gg, base, delta):
    """rhs AP [128, 2(pair), 3(oh), 96(ow*b)] into h1b [128, N1F]."""
    v = h1b[:]
    ap = [list(v.ap[0]), [delta, 2], [448, 3], [1, 96]]
    return bass.AP(v.tensor, v.offset + gg * 1344 + base, ap)


def _build_bass():
    nc = bacc.Bacc("TRN2", target_bir_lowering=False, debug=False,
                   num_devices=NCORES)

    pext_d = nc.dram_tensor("pext", [50, N1F], BF16, kind="ExternalInput").ap()
    w1pr_d = nc.dram_tensor("w1pr", [50, DP], BF16, kind="ExternalInput").ap()
    cub_d = nc.dram_tensor("cub", [75, 32], BF16, kind="ExternalInput").ap()
    w1tr_d = nc.dram_tensor("w1tr", [75, DP], FP8, kind="ExternalInput").ap()
    w1p_d = nc.dram_tensor("w1p", [128, DT * 32], BF16, kind="ExternalInput").ap()
    diag_d = nc.dram_tensor("diag", [128, DT * 5 * 2 * 128], FP8,
                            kind="ExternalInput").ap()
    w3_d = nc.dram_tensor("w3", [128, DT * 36], F32, kind="ExternalInput").ap()
    bn_d = nc.dram_tensor("bn", [128, 60], F32, kind="ExternalInput").ap()
    wct_d = nc.dram_tensor("wct", [128, DT * 10], BF16, kind="ExternalInput").ap()
    out_d = nc.dram_tensor("sims", [B, 10], F32, kind="ExternalOutput").ap()

    with tile.TileContext(nc) as tc:
        with (
            tc.tile_pool(name="const", bufs=1) as const,
            tc.tile_pool(name="work", bufs=3) as work,
            tc.tile_pool(name="stat", bufs=2) as stat,
            tc.tile_pool(name="pc1", bufs=2, space="PSUM") as pc1,
            tc.tile_pool(name="pc2", bufs=2, space="PSUM") as pc2,
        ):
            # ---------------- DMA in (spread across engine queues so
            # the rings + transfers overlap; every engine is idle at t=0).
            # scalar queue: the small tensors gating the V phase + bias1;
            # sync/gpsimd: the two pext halves + w1pr + diag.
            cub = const.tile([75, 32], BF16)
            nc.sync.dma_start(out=cub[:], in_=cub_d[:])
            w1tr = const.tile([75, DP], FP8)
            nc.sync.dma_start(out=w1tr[:], in_=w1tr_d[:])
            w1pr = const.tile([50, DP], BF16)
            nc.sync.dma_start(out=w1pr[:], in_=w1pr_d[:])
            pext = const.tile([50, N1F], BF16)
            nc.sync.dma_start(out=pext[:, 0:1536], in_=pext_d[:, 0:1536])
            nc.gpsimd.dma_start(out=pext[:, 1536:N1F],
                                in_=pext_d[:, 1536:N1F])
            bnt = const.tile([128, 6, DT], F32)
            nc.scalar.dma_start(
                out=bnt[:].rearrange("p l t -> p (l t)"), in_=bn_d[:])
            w1p = const.tile([128, DT, 32], BF16)
            nc.scalar.dma_start(
                out=w1p[:].rearrange("p t k -> p (t k)"), in_=w1p_d[:])
            diag = const.tile([128, DT, 5, 2, 128], FP8)
            DTL = 5 * 2 * 128
            for t in range(DT):
                nc.gpsimd.dma_start(
                    out=diag[:, t].rearrange("p k i c -> p (k i c)"),
                    in_=diag_d[:, t * DTL:(t + 1) * DTL])
            w3t = const.tile([128, DT, 36], F32)
            nc.gpsimd.dma_start(
                out=w3t[:].rearrange("p t s -> p (t s)"), in_=w3_d[:])
            wcs = const.tile([128, DT, 10], BF16)
            nc.gpsimd.dma_start(
                out=wcs[:].rearrange("p t c -> p (t c)"), in_=wct_d[:])

            epsc = const.tile([128, 1], F32)
            nc.vector.memset(epsc[:], EPS)

            # beMrgT[:, L, t] = (beta_L - 0.5) / gamma_L   (gamma > 0)
            rg = const.tile([128, 3, DT], F32)
            nc.vector.reciprocal(rg[:], bnt[:, 0::2, :])
            beMrgT = const.tile([128, 3, DT], F32)
            nc.vector.tensor_scalar(beMrgT[:], bnt[:, 1::2, :], -0.5, None,
                                    ALU.add)
            nc.vector.tensor_tensor(beMrgT[:], beMrgT[:], rg[:], ALU.mult)
            negBeMrgT = const.tile([128, 3, DT], F32)
            nc.vector.tensor_scalar(negBeMrgT[:], beMrgT[:], -1.0, None,
                                    ALU.mult)

            h3b_all = const.tile([128, DT, B], BF16)
            bias1 = const.tile([128, DT], F32)

            # ---------------- pipelined tile loop (defs) ----------------
            h1bs = {}

            def conv1_mm_thunks(t):
                lhs = w1pr[:, t * 128:(t + 1) * 128]
                thirds = []
                mms = []
                o = 0
                for chunks in CH3:
                    p1 = pc1.tile([128, 2, 512], F32, tag="c1")
                    for ci, cs in enumerate(chunks):
                        def mk(p1=p1, ci=ci, cs=cs, o=o):
                            nc.tensor.matmul(p1[:, ci, 0:cs], lhsT=lhs,
                                             rhs=pext[:, o:o + cs],
                                             start=True, stop=True)
                        mms.append(mk)
                        o += cs
                    thirds.append(p1)
                return thirds, mms

            def conv1_mm(t):
                thirds, mms = conv1_mm_thunks(t)
                for fn in mms:
                    fn()
                return thirds

            def conv1_bin(t, thirds):
                # h1b padded to 3072: each psum third binarizes in ONE ACT
                # (cols beyond 2912 are junk, never read by conv2; the last
                # third reads 160 unwritten psum cols whose sign lands in
                # the junk zone)
                h1b = work.tile([128, 3072], FP8, tag="h1b")
                b1 = bias1[:, t:t + 1]
                for k, p1 in enumerate(thirds):
                    nc.scalar.activation(
                        h1b[:, k * 1024:(k + 1) * 1024].rearrange(
                            "p (a b) -> p a b", a=2, b=512),
                        p1[:], ACTF.Sign, bias=b1, scale=1.0)
                h1bs[t] = h1b

            # ---------------- phase V: BN1 stats matmuls ----------------
            # V[c, t*32+k] = w1s_t^T @ [C|u] with the 3 bf16 parts stacked
            # on K (75 rows); one 32-col matmul per tile into bank 0.
            pv = pc2.tile([128, 2, 512], F32, tag="c2")
            pvv = pv[:, 0, 0:DT * 32].rearrange("p (t k) -> p t k", t=DT, k=32)
            for t in range(DT):
                nc.tensor.matmul(pvv[:, t, :],
                                 lhsT=w1tr[:, t * 128:(t + 1) * 128],
                                 rhs=cub[:], start=True, stop=True)

            # conv1(0) matmuls after the V phase on the PE queue: V's
            # inputs (cub/w1tr, scalar queue) land before pext, so the PE
            # starts on V while pext streams in.
            thirds0 = conv1_mm(0)

            # ---------------- batched BN1 algebra (DVE) ----------------
            H = const.tile([128, DT, 32], F32)
            nc.vector.tensor_tensor(H[:], pvv, w1p[:], ALU.mult)
            m2s = const.tile([128, DT], F32)
            nc.vector.tensor_reduce(m2s[:], H[:], AX.X, ALU.add)
            mean1 = const.tile([128, DT], F32)
            nc.vector.tensor_scalar(mean1[:], pvv[:, :, 25], 1.0 / N1,
                                    None, ALU.mult)
            mm1 = const.tile([128, DT], F32)
            nc.vector.tensor_tensor(mm1[:], mean1[:], mean1[:], ALU.mult)
            var1 = const.tile([128, DT], F32)
            nc.vector.scalar_tensor_tensor(var1[:], m2s[:], 1.0 / N1, mm1[:],
                                           ALU.mult, ALU.subtract)
            sd1 = const.tile([128, DT], F32)
            nc.scalar.activation(sd1[:], var1[:], ACTF.Sqrt, bias=epsc[:],
                                 scale=1.0)
            nc.vector.tensor_tensor(bias1[:], sd1[:], beMrgT[:, 0, :],
                                    ALU.mult)
            nc.vector.tensor_tensor(bias1[:], bias1[:], mean1[:], ALU.subtract)
            negb1 = const.tile([128, DT], F32)
            nc.vector.tensor_scalar(negb1[:], bias1[:], -1.0, None, ALU.mult)

            conv1_bin(0, thirds0)

            # ---------------- pipelined tile loop ----------------
            def conv2_front(t, c1_mms=None):
                """Gating path: conv2 matmuls -> stats -> thr2 -> is_gt.

                conv2 is LDWEIGHTS-bound (229ns load vs 120ns exec); conv1
                is exec-bound (427ns).  Interleaving conv1(t+1) chunks
                between conv2(t) pairs hides every conv2 weight load
                behind a conv1 exec on the in-order PE queue."""
                h1b = h1bs.pop(t)
                c1_mms = list(c1_mms or [])
                p2 = pc2.tile([128, 2, 512], F32, tag="c2")
                for gg in range(2):
                    for pk, (ka, kb, base, delta) in enumerate(PAIRS):
                        if c1_mms:
                            c1_mms.pop(0)()
                        nc.tensor.matmul(p2[:, gg, 0:G2],
                                         lhsT=diag[:, t, pk],
                                         rhs=_pair_ap(h1b, gg, base, delta),
                                         start=(pk == 0), stop=(pk == 4),
                                         perf_mode=DR)
                for fn in c1_mms:
                    fn()
                # the whole stats->threshold->is_gt chain gates conv2(t+1)
                # via the p2 buffer; raise its scheduler priority so it
                # sorts ahead of the previous tile's conv3 tail ops.
                with tc.high_priority(offset=25):
                    st2 = stat.tile([128, 2, 6], F32, tag="st2")
                    for gg in range(2):
                        nc.vector.bn_stats(st2[:, gg, :], p2[:, gg, 0:G2])
                    mv2 = stat.tile([128, 2], F32, tag="mv2")
                    nc.vector.bn_aggr(mv2[:], st2[:])
                    sq2 = stat.tile([128, 1], F32, tag="sq2")
                    nc.scalar.activation(sq2[:], mv2[:, 1:2], ACTF.Sqrt,
                                         bias=epsc[:], scale=1.0)
                    # threshold: h2 = (y > thr2) in {0,1}; BN3 absorbs the
                    # per-channel affine change of conv3 preacts.
                    thr2 = stat.tile([128, 1], F32, tag="thr2")
                    nc.vector.scalar_tensor_tensor(
                        thr2[:], sq2[:], negBeMrgT[:, 1, t:t + 1],
                        mv2[:, 0:1], ALU.mult, ALU.add)
                    h2b = work.tile([128, 2, G2], BF16, tag="h2b")
                    nc.vector.tensor_scalar(h2b[:], p2[:, :, 0:G2], thr2[:],
                                            None, ALU.is_gt)
                return h2b

            def conv3_tail(t, h2b):
                """Lagging path: conv3 + BN3 + binarize3 (not tile-gating)."""
                h2sb = h2b[:].rearrange(
                    "p g (x b) -> p g x b", b=B).transpose([0, 3, 1, 2])
                tmp3 = work.tile([128, B, 36], F32, tag="tmp3")
                mul_eng = nc.vector if t == DT - 1 else nc.gpsimd
                mul_eng.tensor_tensor(
                    tmp3[:].rearrange("p b (g x) -> p b g x", g=2, x=18), h2sb,
                    w3t[:, t, :].unsqueeze(1).broadcast_to(
                        [128, B, 36]).rearrange("p b (g x) -> p b g x",
                                                g=2, x=18),
                    ALU.mult)
                h3pre = stat.tile([128, B], F32, tag="h3pre")
                nc.vector.tensor_reduce(h3pre[:], tmp3[:], AX.X, ALU.add)
                st3 = stat.tile([128, 6], F32, tag="st3")
                nc.vector.bn_stats(st3[:], h3pre[:])
                mv3 = stat.tile([128, 2], F32, tag="mv3")
                nc.vector.bn_aggr(mv3[:], st3[:])
                sq3 = stat.tile([128, 1], F32, tag="sq3")
                nc.scalar.activation(sq3[:], mv3[:, 1:2], ACTF.Sqrt,
                                     bias=epsc[:], scale=1.0)
                # h3 in {0,1} via DVE is_gt (host corrects the classifier:
                # sims = 2*sims01 - colsum(sign(Wc))/sqrt(D))
                thr3 = stat.tile([128, 1], F32, tag="bias3")
                nc.vector.scalar_tensor_tensor(
                    thr3[:], sq3[:], negBeMrgT[:, 2, t:t + 1], mv3[:, 0:1],
                    ALU.mult, ALU.add)
                nc.vector.tensor_scalar(h3b_all[:, t, :], h3pre[:], thr3[:],
                                        None, ALU.is_gt)

            # Emission order per tile:
            #   c1mm(t+1)  [PE ahead of c2mm(t)]
            #   c2front(t) [gating: mm, stats, sq2, thr2, is_gt]
            #   c1bin(t+1) [ACT right after sq2(t), before sq3(t)/h3(t)]
            #   c3tail(t)  [lagging chain, never gates tile t+1]
            for it in range(DT):
                nxt, mms = (None, None)
                if it + 1 < DT:
                    nxt, mms = conv1_mm_thunks(it + 1)
                h2b = conv2_front(it, mms)
                if nxt is not None:
                    conv1_bin(it + 1, nxt)
                conv3_tail(it, h2b)

            # ---------------- classifier (partial sims per core) ----------
            pcls = pc2.tile([128, 2, 512], F32, tag="c2")
            for t in range(DT):
                nc.tensor.matmul(pcls[0:B, 0, 0:10], lhsT=h3b_all[:, t, :],
                                 rhs=wcs[:, t, :],
                                 start=(t == 0), stop=(t == DT - 1))
            sims_sb = stat.tile([B, 10], F32, tag="sims_sb")
            nc.scalar.mul(sims_sb[:], pcls[0:B, 0, 0:10],
                          1.0 / np.sqrt(np.float32(D)))
            nc.sync.dma_start(out=out_d[:], in_=sims_sb[:])

    nc.compile()
    return nc


def get_nc():
    if "nc" not in _CACHE:
        _CACHE["nc"] = _build_bass()
    return _CACHE["nc"]


def prep_inputs(x, W1, b1, g1, be1, W2, b2, g2, be2, W3, b3, g3, be3, Wc):
    """Host-side layout/sharding prep.

    Conv biases b1/b2/b3 are dropped: training-mode BN is invariant to a
    per-channel additive constant before normalization.
    """
    import ml_dtypes
    f = np.float32
    bf = ml_dtypes.bfloat16
    f8 = ml_dtypes.float8_e4m3
    f8w = ml_dtypes.float8_e5m2

    xp = np.zeros((B, 30, 30), f)
    xp[:, 1:29, 1:29] = np.asarray(x, f)[:, 0]
    win = sliding_window_view(xp, (5, 5), axis=(1, 2))[:, ::2, ::2]
    # field-order columns [h][w-parity][w//2][b] (batch innermost, strides
    # h:224 par:112 wp:16 b:1) with 1 pad col-group per h row
    Pf = np.zeros((25, H1, 2, 7, B), f)
    wv = win.transpose(3, 4, 0, 1, 2)          # [5,5,b,h,w] -> flat 25
    wv = wv.reshape(25, B, H1, H1)
    Pf[:, :, 0, :, :] = wv[:, :, :, 0::2].transpose(0, 2, 3, 1)  # w even (7)
    Pf[:, :, 1, :6, :] = wv[:, :, :, 1::2].transpose(0, 2, 3, 1)  # w odd (6)
    P = Pf.reshape(25, N1F)

    # hi/lo bf16 streams stacked on K (exact to 2^-16)
    phi = P.astype(bf)
    plo = (P - phi.astype(f)).astype(bf)
    pext = np.zeros((50, N1F), bf)
    pext[0:25] = phi
    pext[25:50] = plo

    # covariance+colsum in 3 bf16 parts stacked on K
    P64 = P.astype(np.float64)
    cu32 = np.concatenate(
        [(P64 @ P64.T).astype(f), P64.sum(1).astype(f)[:, None]], 1)  # [25,26]
    cub = np.zeros((75, 32), bf)
    rem2 = cu32.astype(np.float64)
    for part in range(3):
        p_ = rem2.astype(f).astype(bf)
        cub[part * 25:(part + 1) * 25, 0:26] = p_
        rem2 = rem2 - p_.astype(np.float64)

    def padrows(a, width, fill=0.0):
        out = np.full((DPAD, width), fill, f)
        out[:D] = np.asarray(a, f).reshape(D, width)
        return out

    w1sp = np.sign(padrows(W1, 25))          # [DPAD, 25] in {-1,0,1}
    w2sp = np.sign(padrows(W2, 9))
    w3p = padrows(W3, 36)
    wcsp = np.sign(padrows(np.asarray(Wc, f).T, 10))
    bn = np.zeros((DPAD, 6), f)
    bn[:, 0::2] = 1.0
    bn[:D, 0] = np.asarray(g1, f)
    bn[:D, 1] = np.asarray(be1, f)
    bn[:D, 2] = np.asarray(g2, f)
    bn[:D, 3] = np.asarray(be2, f)
    bn[:D, 4] = np.asarray(g3, f)
    bn[:D, 5] = np.asarray(be3, f)

    def tmajor(a, width):
        # [1280, width] -> [128, DT, width]
        return np.ascontiguousarray(
            a.reshape(DT, 128, width).transpose(1, 0, 2))

    in_maps = []
    for c in range(NCORES):
        sl = slice(c * DP, (c + 1) * DP)
        w1sl = w1sp[sl]                       # [1280, 25]
        w1t = w1sl.T                          # [25, 1280]
        # bf16 K-stacked (2x) conv1 weights
        w1pr = np.zeros((50, DP), bf)
        w1pr[0:25] = w1t.astype(bf)
        w1pr[25:50] = w1t.astype(bf)
        # K-stacked (3x) fp8 weights for the BN1 stats matmuls
        w1tr = np.zeros((75, DP), f8)
        for j in range(3):
            w1tr[j * 25:(j + 1) * 25] = w1t.astype(f8)
        # w1s t-major padded to 32 for the H product
        w1pt = np.zeros((128, DT, 32), bf)
        w1pt[:, :, 0:25] = tmajor(w1sl, 25)
        # conv2 diagonal pair weights
        w2tm = tmajor(w2sp[sl], 9)            # [128, DT, 9]
        diagp = np.zeros((128, DT, 5, 2, 128), f8)
        idx = np.arange(128)
        for pk, (ka, kb, base, delta) in enumerate(PAIRS):
            diagp[idx, :, pk, 0, idx] = w2tm[:, :, ka].astype(f8)
            if kb is not None:
                diagp[idx, :, pk, 1, idx] = w2tm[:, :, kb].astype(f8)
        in_maps.append({
            "pext": pext,
            "w1pr": w1pr,
            "cub": cub,
            "w1tr": w1tr,
            "w1p": w1pt.reshape(128, DT * 32),
            "diag": diagp.reshape(128, DT * 5 * 2 * 128),
            "w3": tmajor(w3p[sl], 36).reshape(128, DT * 36),
            # bn: [128, 6, DT] L-major, t-minor
            "bn": np.ascontiguousarray(
                tmajor(bn[sl], 6).transpose(0, 2, 1)).reshape(128, 60),
            "wct": tmajor(wcsp[sl], 10).astype(bf).reshape(128, DT * 10),
        })
    return in_maps


def kernel(**inputs) -> np.ndarray:
    from concourse.bass_utils import run_bass_kernel_spmd
    nc = get_nc()
    in_maps = prep_inputs(**inputs)
    res = run_bass_kernel_spmd(nc, in_maps, list(range(NCORES)))
    acc = np.zeros((B, 10), np.float64)
    for r in res.results:
        acc += 2.0 * np.asarray(r["sims"], np.float64)
    wcs_cols = np.sign(np.asarray(inputs["Wc"], np.float64)).sum(1)
    acc -= wcs_cols[None, :] / np.sqrt(np.float64(D))
    return acc.astype(np.float32)
